# revision 16
# baseline (speedup 1.0000x reference)
"""CrossAttention2D Trainium2 kernel.

Reference computation (per batch b, with C=256, HW=64*64=4096):
  q = wq @ x_q + bq        [C, HW]   (1x1 conv == channel-mixing GEMM)
  k = wk @ x_k + bk        [C, HW]
  v = wv @ x_v + bv        [C, HW]
  S = q^T k                [HW, HW]
  P = softmax(S, axis=-1)
  out = (P @ v^T)^T        [C, HW]

Sharding: data-parallel over batch B=8 across the 8 NeuronCores (one
batch per core). Each core runs an identical Bass program on its own
batch slice; no collectives.

Per-core dataflow (all matmuls in float32r: full-rate on the PE with
~1e-4 relative error):
  - Projections produce Q,K in [o, n] layout and V transposed in
    [m, o] layout (so the attention*V matmul needs no transpose of P).
  - S^T[m, n] = sum_o K[o, m] Q[o, n] computed m-block by m-block;
    exp() on the scalar engine moves PSUM->SBUF.
  - Softmax denominators: ones-vector matmuls accumulate column sums
    of exp(S^T) on the tensor engine (partition-axis reduction).
  - out^T[o, n] accumulates sum_m V[m, o] expST[m, n] in PSUM.
  - Tail: transpose out^T -> [n, o], scale rows by 1/denom (free fused
    into the scalar-engine PSUM->SBUF copy), transpose back, DMA out.
"""

import numpy as np

import concourse.bacc as bacc
import concourse.tile as tile
from concourse import mybir
from concourse.bass_utils import run_bass_kernel_spmd
from concourse.masks import make_identity

F32 = mybir.dt.float32
F32R = mybir.dt.float32r

B, C, H, W = 8, 256, 64, 64
HW = H * W            # 4096
NT = 512              # n-tile width (max fp32 moving operand / PSUM bank)
N_TILES = HW // NT    # 8
MC = HW // 128        # 32 m-chunks of 128
OC = C // 128         # 2 o-chunks of 128
NB = NT // 128        # 4 n-blocks per n-tile

_CACHE = {}


def _build(repeat=1):
    """repeat>1 wraps the attention phase in a hardware loop — used only by
    the benchmarking harness to measure per-iteration HW time via wall-clock
    deltas (the container has no NTFF profiling hook)."""
    nc = bacc.Bacc("TRN2", target_bir_lowering=False, debug=False, num_devices=B)

    xq_d = nc.dram_tensor("xq", [C, HW], F32R, kind="ExternalInput")
    xk_d = nc.dram_tensor("xk", [C, HW], F32R, kind="ExternalInput")
    xv_d = nc.dram_tensor("xv", [C, HW], F32R, kind="ExternalInput")
    wq_d = nc.dram_tensor("wqT", [C, C], F32R, kind="ExternalInput")
    wk_d = nc.dram_tensor("wkT", [C, C], F32R, kind="ExternalInput")
    wv_d = nc.dram_tensor("wvT", [C, C], F32R, kind="ExternalInput")
    bq_d = nc.dram_tensor("bq2", [1, C], F32R, kind="ExternalInput")
    bk_d = nc.dram_tensor("bk2", [1, C], F32R, kind="ExternalInput")
    bv_d = nc.dram_tensor("bv2", [1, C], F32R, kind="ExternalInput")
    out_d = nc.dram_tensor("out", [C, HW], F32, kind="ExternalOutput")

    with tile.TileContext(nc) as tc:
        with (
            tc.tile_pool(name="persist", bufs=1) as persist,
            tc.tile_pool(name="stage", bufs=3) as stage,
            tc.tile_pool(name="work", bufs=4) as work,
            tc.tile_pool(name="tail", bufs=3) as tail,
            tc.tile_pool(name="ps_s", bufs=3, space="PSUM") as ps_s,
            tc.tile_pool(name="ps_av", bufs=2, space="PSUM") as ps_av,
            tc.tile_pool(name="ps_tp", bufs=2, space="PSUM") as ps_tp,
            tc.tile_pool(name="ps_dn", bufs=1, space="PSUM") as ps_dn,
        ):
            # ---- constants ----
            ident32 = persist.tile([128, 128], F32, tag="ident32")
            make_identity(nc, ident32)
            ident = persist.tile([128, 128], F32R, tag="ident")
            nc.vector.tensor_copy(ident, ident32)

            ones32c = persist.tile([128, 1], F32, tag="ones32c")
            nc.vector.memset(ones32c, 1.0)
            ones_col = persist.tile([128, 1], F32R, tag="ones_col")
            nc.vector.tensor_copy(ones_col, ones32c)
            ones32r = persist.tile([1, NT], F32, tag="ones32r")
            nc.vector.memset(ones32r, 1.0)
            ones_row = persist.tile([1, NT], F32R, tag="ones_row")
            nc.vector.tensor_copy(ones_row, ones32r)
            ones11 = ones32r[0:1, 0:1]  # fp32, for the K=1/N=1 scatter matmuls

            # ---- weights / biases ----
            wq_sb = persist.tile([128, OC, C], F32R, tag="wq")
            wk_sb = persist.tile([128, OC, C], F32R, tag="wk")
            wv_sb = persist.tile([128, OC, C], F32R, tag="wv")
            for cc in range(OC):
                nc.sync.dma_start(wq_sb[:, cc, :], wq_d[cc * 128:(cc + 1) * 128, :])
                nc.sync.dma_start(wk_sb[:, cc, :], wk_d[cc * 128:(cc + 1) * 128, :])
                nc.sync.dma_start(wv_sb[:, cc, :], wv_d[cc * 128:(cc + 1) * 128, :])
            bq_sb = persist.tile([1, C], F32R, tag="bq")
            bk_sb = persist.tile([1, C], F32R, tag="bk")
            bv_sb = persist.tile([1, C], F32R, tag="bv")
            nc.sync.dma_start(bq_sb, bq_d[:, :])
            nc.sync.dma_start(bk_sb, bk_d[:, :])
            nc.sync.dma_start(bv_sb, bv_d[:, :])

            # ---- projections ----
            q_sb = persist.tile([128, OC, HW], F32R, tag="q")
            k_sb = persist.tile([128, OC, HW], F32R, tag="k")
            v_sb = persist.tile([128, MC, C], F32R, tag="v")

            def project_qk(x_d, w_sb, b_sb, dst, nt):
                # bias applied as a rank-1 K=1 matmul (bias[o] x ones[n]) so
                # the scalar engine runs nothing but Exp in steady state
                sl = slice(nt * NT, (nt + 1) * NT)
                x_t = stage.tile([128, OC, NT], F32R, tag="xstage", name="x_t")
                for cc in range(OC):
                    nc.sync.dma_start(x_t[:, cc, :], x_d[cc * 128:(cc + 1) * 128, sl])
                for oc in range(OC):
                    ps = ps_s.tile([128, NT], F32, tag="st", name="ps")
                    for cc in range(OC):
                        nc.tensor.matmul(
                            ps,
                            w_sb[:, cc, oc * 128:(oc + 1) * 128],
                            x_t[:, cc, :],
                            start=(cc == 0),
                            stop=False,
                        )
                    nc.tensor.matmul(
                        ps,
                        b_sb[0:1, oc * 128:(oc + 1) * 128],
                        ones_row,
                        start=False,
                        stop=True,
                    )
                    nc.vector.tensor_copy(dst[:, oc, sl], ps)

            # K first (attention needs all of K), then V, then Q streamed
            # tile-by-tile inside the attention loop.
            for nt in range(N_TILES):
                project_qk(xk_d, wk_sb, bk_sb, k_sb, nt)
            for nt in range(N_TILES):
                # V in transposed layout: V[m, o] = sum_c x_v[c, m] wvT[c, o] + bv[o]
                sl = slice(nt * NT, (nt + 1) * NT)
                x_t = stage.tile([128, OC, NT], F32R, tag="xstage", name="x_t")
                for cc in range(OC):
                    nc.sync.dma_start(x_t[:, cc, :], xv_d[cc * 128:(cc + 1) * 128, sl])
                for sub in range(NB):
                    mb = nt * NB + sub
                    psv = ps_av.tile([128, NT], F32, tag="av", name="psv")
                    msl = slice(sub * 128, (sub + 1) * 128)
                    nc.tensor.matmul(
                        psv[:, 0:C], x_t[:, 0, msl], wv_sb[:, 0, :],
                        start=True, stop=False,
                    )
                    nc.tensor.matmul(
                        psv[:, 0:C], x_t[:, 1, msl], wv_sb[:, 1, :],
                        start=False, stop=False,
                    )
                    nc.tensor.matmul(
                        psv[:, 0:C], ones_row[0:1, 0:128], bv_sb,
                        start=False, stop=True,
                    )
                    nc.vector.tensor_copy(v_sb[:, mb, :], psv[:, 0:C])

            # ---- attention ----
            rd_all = persist.tile([128, MC], F32, tag="rd")

            import contextlib

            loop_ctx = (
                tc.For_i(0, repeat, 1) if repeat > 1 else contextlib.nullcontext()
            )
            with loop_ctx:
              for nt in range(N_TILES):
                project_qk(xq_d, wq_sb, bq_sb, q_sb, nt)
                sl = slice(nt * NT, (nt + 1) * NT)
                dn_ps = ps_dn.tile([1, NT], F32, tag="dn")
                av_ps = [
                    ps_av.tile([128, NT], F32, tag="av", name=f"avps{oc}")
                    for oc in range(OC)
                ]
                for mb in range(MC):
                    ps = ps_s.tile([128, NT], F32, tag="st")
                    msl = slice(mb * 128, (mb + 1) * 128)
                    for oc in range(OC):
                        nc.tensor.matmul(
                            ps,
                            k_sb[:, oc, msl],
                            q_sb[:, oc, sl],
                            start=(oc == 0),
                            stop=(oc == OC - 1),
                        )
                    est = work.tile([128, NT], F32R, tag="expst")
                    nc.scalar.activation(
                        out=est, in_=ps, func=mybir.ActivationFunctionType.Exp
                    )
                    nc.tensor.matmul(
                        dn_ps, ones_col, est,
                        start=(mb == 0), stop=(mb == MC - 1),
                    )
                    for oc in range(OC):
                        nc.tensor.matmul(
                            av_ps[oc],
                            v_sb[:, mb, oc * 128:(oc + 1) * 128],
                            est,
                            start=(mb == 0),
                            stop=(mb == MC - 1),
                        )

                # denominators -> per-partition reciprocal columns
                # (scatter matmuls run in plain fp32: f32r forbids N==1)
                dn_sb = tail.tile([1, NT], F32, tag="dnsb")
                nc.vector.tensor_copy(dn_sb, dn_ps)
                rd_ps = ps_tp.tile([128, NB], F32, tag="tp")
                for j in range(NB):
                    nc.tensor.matmul(
                        rd_ps[:, j:j + 1],
                        dn_sb[0:1, j * 128:(j + 1) * 128],
                        ones11,
                        start=True, stop=True,
                    )
                nc.vector.reciprocal(rd_all[:, nt * NB:(nt + 1) * NB], rd_ps)

                # tail: normalize + emit [C, HW] layout (copies on DVE so the
                # scalar engine stays exclusively on Exp)
                avt_sb = tail.tile([128, OC, NT], F32R, tag="avtsb")
                for oc in range(OC):
                    nc.vector.tensor_copy(avt_sb[:, oc, :], av_ps[oc])
                for j in range(NB):
                    nb = nt * NB + j
                    jsl = slice(j * 128, (j + 1) * 128)
                    for oc in range(OC):
                        t1 = ps_tp.tile([128, 128], F32R, tag="tp")
                        nc.tensor.transpose(t1, avt_sb[:, oc, jsl], ident)
                        no_sb = tail.tile([128, 128], F32R, tag="nosb")
                        nc.vector.tensor_scalar_mul(
                            no_sb, t1.bitcast(F32), rd_all[:, nb:nb + 1]
                        )
                        t2 = ps_tp.tile([128, 128], F32R, tag="tp")
                        nc.tensor.transpose(t2, no_sb, ident)
                        ot_sb = tail.tile([128, 128], F32, tag="otsb")
                        nc.vector.tensor_copy(ot_sb, t2.bitcast(F32))
                        nc.sync.dma_start(
                            out_d[oc * 128:(oc + 1) * 128, nb * 128:(nb + 1) * 128],
                            ot_sb,
                        )

    nc.compile()
    return nc


def kernel(query, key, value, wq, bq, wk, bk, wv, bv):
    if "nc" not in _CACHE:
        _CACHE["nc"] = _build()
    nc = _CACHE["nc"]

    query = np.ascontiguousarray(query, dtype=np.float32).reshape(B, C, HW)
    key = np.ascontiguousarray(key, dtype=np.float32).reshape(B, C, HW)
    value = np.ascontiguousarray(value, dtype=np.float32).reshape(B, C, HW)
    shared = {
        "wqT": np.ascontiguousarray(np.asarray(wq, np.float32).T),
        "wkT": np.ascontiguousarray(np.asarray(wk, np.float32).T),
        "wvT": np.ascontiguousarray(np.asarray(wv, np.float32).T),
        "bq2": np.asarray(bq, np.float32).reshape(1, C),
        "bk2": np.asarray(bk, np.float32).reshape(1, C),
        "bv2": np.asarray(bv, np.float32).reshape(1, C),
    }
    in_maps = [
        {"xq": query[b], "xk": key[b], "xv": value[b], **shared} for b in range(B)
    ]
    res = run_bass_kernel_spmd(nc, in_maps, core_ids=list(range(B)))
    out = np.stack([res.results[b]["out"] for b in range(B)])
    return out.reshape(B, C, H, W)
